# revision 1
# baseline (speedup 1.0000x reference)
"""Trainium2 Bass kernel for DBRX attention (B=2, S=2048, D=4096, 32 q-heads,
8 kv-heads GQA, causal, RoPE, fp32 reference), 8-way head-tensor-parallel.

Sharding: core c owns q-heads 4c..4c+3 and kv-head c (GQA groups stay
aligned). Each core computes its 512-dim slice of attention output, then a
full-token out_proj partial with its 512-row slice of out_w; the host sums
the 8 partials (the "all-reduce after out_proj" of the hint, done at gather
time).

Performance notes (v2):
  - ALL matmul operands are fp16: enables FWL (fast weight load, disabled
    for fp32) so LDWEIGHTS overlaps matmuls via the PE reorder window, and
    removes the fp32r 4x penalty on <256-col matmuls. PSUM stays fp32.
  - q stays RESIDENT in SBUF (fp16 halves the footprint) - no DRAM
    spill/reload between projection and attention.
  - softmax row sums via an all-ones [128,128] stationary matmul: the sum
    lands broadcast across all 128 psum partitions, so no separate
    rank-1 "broadcast 1/l" matmul is needed.
  - causal diagonal blocks: exp first (no mask), then multiply by a 0/1
    upper-triangle tile on the (otherwise idle) Pool/GpSimd engine.
  - softmax without a max pass: exp(S/sqrt(d) - C) with constant C; exact
    for any C (shift invariance); pt is fp16 so C=6 keeps the dominant
    weights in fp16 normal range.
"""

import math
import os
import sys

import numpy as np

for _p in ("/root/.axon_site/_ro/trn_rl_repo", "/opt/trn_rl_repo"):
    if os.path.isdir(_p) and _p not in sys.path:
        sys.path.append(_p)

import concourse.bass as bass
import concourse.tile as tile
from concourse import bacc, mybir
from concourse.bass_utils import run_bass_kernel_spmd

F32 = mybir.dt.float32
F16 = mybir.dt.float16


def R(ap):
    return ap

N_CORES = 8
DH = 128          # head dim
HPC = 4           # q heads per core
NF = HPC + 2      # qkv feature tiles of 128 per core (4 q + 1 k + 1 v)
CLIP = 8.0
ROPE_THETA = 500000.0
ISQ = 1.0 / math.sqrt(DH)
EXP_C = 6.0       # constant softmax shift (exact for any value; see header)


def build_program(B, S, D, causal=True, debug=False, reps=1):
    """Build the single-core Bass program (same program on all 8 cores)."""
    T = B * S                  # total tokens
    KB = D // 128              # contraction chunks for the projections
    SKB = S // 128             # k blocks per batch in attention
    MT = T // 128              # token m-tiles for out_proj
    OFW = min(D, 1024)         # out-feature psum group width
    OFH = D // OFW
    OW2 = min(D, 2048)         # out eviction/DMA group width
    OH2 = D // OW2

    nc = bacc.Bacc(
        "TRN2",
        target_bir_lowering=False,
        debug=debug,
        num_devices=N_CORES,
    )

    hid = nc.dram_tensor("hidden_t", [D, T], F16, kind="ExternalInput")
    wqkv = nc.dram_tensor("wqkv_t", [D, NF * 128], F16, kind="ExternalInput")
    outw = nc.dram_tensor("outw_t", [HPC * DH, D], F16, kind="ExternalInput")
    cos_d = nc.dram_tensor("cos_t", [DH, T], F16, kind="ExternalInput")
    sin_d = nc.dram_tensor("sin_t", [DH, T], F32, kind="ExternalInput")
    rot_d = nc.dram_tensor("rot_t", [DH, DH], F16, kind="ExternalInput")
    t01_d = nc.dram_tensor("trimask01", [128, 128], F16, kind="ExternalInput")
    idn_d = nc.dram_tensor("identity", [128, 128], F16, kind="ExternalInput")
    out_d = nc.dram_tensor("out_partial", [MT, OH2, 128, OW2], F16,
                           kind="ExternalOutput")

    Exp = mybir.ActivationFunctionType.Exp
    Copy = mybir.ActivationFunctionType.Copy
    Alu = mybir.AluOpType

    from contextlib import ExitStack

    with ExitStack() as ctx:
        tc = ctx.enter_context(tile.TileContext(nc))
        PSUM = bass.MemorySpace.PSUM
        constp = ctx.enter_context(tc.tile_pool(name="const", bufs=1))
        # one PSUM pool, 4 tags x 2 banks, multiplexed across phases
        psp = ctx.enter_context(tc.tile_pool(name="psp", bufs=1, space=PSUM))

        # constants (loaded on the Act HWDGE queue so they don't delay the
        # sync queue's first hidden/weight loads)
        t01 = constp.tile([128, 128], F16, tag="t01", name="t01")
        nc.scalar.dma_start(t01[:], t01_d.ap())
        idn = constp.tile([128, 128], F16, tag="idn", name="idn")
        nc.scalar.dma_start(idn[:], idn_d.ap())
        rott = constp.tile([DH, DH], F16, tag="rot", name="rot")
        nc.scalar.dma_start(rott[:], rot_d.ap())
        ones128 = constp.tile([128, 128], F16, tag="ones", name="ones")
        nc.vector.memset(ones128[:], 1.0)
        cbias = constp.tile([128, 1], F32, tag="cbias", name="cbias")
        nc.vector.memset(cbias[:], -EXP_C)

        if reps > 1:
            rep_cm = tc.For_i(0, reps, 1)
            rep_cm.__enter__()

        k_t = [None] * B   # [128, S] RoPE'd K, d-major, fp16
        v_sb = [None] * B  # [128, SKB, 128] V, token-major, fp16

        with ExitStack() as kvctx:
            kvp = kvctx.enter_context(tc.tile_pool(name="kv", bufs=2))
            qresp = kvctx.enter_context(tc.tile_pool(name="qres", bufs=1))
            # resident RoPE'd q for all batches/heads [128, B*HPC, S]
            q_sb = qresp.tile([128, B * HPC, S], F16, tag="q", name="q")

            # ============ phase 1: QKV + clip + RoPE (both batches) ========
            with ExitStack() as qctx:
                wqp = qctx.enter_context(tc.tile_pool(name="wq", bufs=1))
                hidp = qctx.enter_context(tc.tile_pool(name="hidp", bufs=2))
                csp = qctx.enter_context(tc.tile_pool(name="cs", bufs=2))
                vtp = qctx.enter_context(tc.tile_pool(name="vt", bufs=1))
                clp = qctx.enter_context(tc.tile_pool(name="clp", bufs=2))
                workp = qctx.enter_context(tc.tile_pool(name="work", bufs=2))

                # resident qkv weights [128, KB, 768] fp16
                w_sb = wqp.tile([128, KB, NF * 128], F16, tag="w", name="w")

                def emit_rope(pend):
                    """RoPE math for a previous 512-token tile (deferred so
                    its rot matmuls slot into the next tile's PE stream)."""
                    b, s0, cso, cl5, cos_c, sin_c, vtr = pend
                    rps_t = psp.tile(
                        [128, 2, 512], F32, tag="p3", name="rotps"
                    )
                    for f in range(NF - 1):
                        cl = cl5[:, f, :]
                        rps = rps_t[:, f % 2, :]
                        nc.tensor.matmul(
                            rps, R(rott[:]), R(cl), start=True, stop=True
                        )
                        t1 = workp.tile([128, 512], F16, tag="t1", name="t1")
                        nc.vector.tensor_tensor(
                            t1[:], cl, cos_c[:, cso:cso + 512], Alu.mult
                        )
                        t2 = workp.tile([128, 512], F16, tag="t2", name="t2")
                        nc.vector.tensor_tensor(
                            t2[:], rps, sin_c[:, cso:cso + 512], Alu.mult
                        )
                        if f < HPC:
                            dest = q_sb[:, b * HPC + f, s0:s0 + 512]
                        else:
                            dest = k_t[b][:, s0:s0 + 512]
                        nc.vector.tensor_tensor(dest, t1[:], t2[:], Alu.add)
                    if vtr is not None:
                        # V -> token-major via PE transpose (end of batch)
                        for to in range(SKB):
                            tps = psp.tile(
                                [128, 128], F16, tag="p3", name="vtps"
                            )
                            nc.tensor.transpose(
                                R(tps[:]),
                                R(vtr[:, to * 128:(to + 1) * 128]),
                                R(idn[:]),
                            )
                            nc.scalar.copy(v_sb[b][:, to, :], tps[:])

                pending = None
                for ti in range(T // 512):
                    t0 = ti * 512
                    b = t0 // S
                    s0 = t0 - b * S
                    if s0 == 0:
                        k_t[b] = kvp.tile([128, S], F16, tag="kt", name="kt")
                        v_sb[b] = kvp.tile(
                            [128, SKB, 128], F16, tag="v", name="v"
                        )
                        v_t = vtp.tile([128, S], F16, tag="vt", name="vt")

                    if t0 % 1024 == 0:
                        cos_c = csp.tile([DH, 1024], F16, tag="cos", name="cos")
                        nc.scalar.dma_start(
                            cos_c[:], cos_d.ap()[:, t0:t0 + 1024]
                        )
                        sin_c = csp.tile([DH, 1024], F32, tag="sin", name="sin")
                        nc.scalar.dma_start(
                            sin_c[:], sin_d.ap()[:, t0:t0 + 1024]
                        )
                    cso = t0 % 1024

                    fps = [
                        psp.tile([128, 2, 512], F32, tag=f"p{i}",
                                 name=f"qkvps{i}")
                        for i in range(NF // 2)
                    ]
                    for kb4 in range(KB // 4):
                        ht = hidp.tile([128, 4, 512], F16, tag="hid", name="hid")
                        nc.sync.dma_start(
                            ht[:],
                            hid.ap()[
                                kb4 * 512:(kb4 + 1) * 512, t0:t0 + 512
                            ].rearrange("(k p) c -> p k c", p=128),
                        )
                        if ti == 0:
                            nc.sync.dma_start(
                                w_sb[:, kb4 * 4:(kb4 + 1) * 4, :],
                                wqkv.ap()[
                                    kb4 * 512:(kb4 + 1) * 512, :
                                ].rearrange("(kb p) f -> p kb f", p=128),
                            )
                        if kb4 < KB // 4 - 1:
                            for ki in range(4):
                                kb = kb4 * 4 + ki
                                for f in range(NF):
                                    nc.tensor.matmul(
                                        fps[f // 2][:, f % 2, :],
                                        R(w_sb[:, kb, f * 128:(f + 1) * 128]),
                                        R(ht[:, ki, :]),
                                        start=(kb == 0),
                                        stop=False,
                                    )
                        else:
                            # last contraction block: f-outer so each psum
                            # group completes (and can be clipped) early
                            for f in range(NF):
                                for ki in range(4):
                                    kb = kb4 * 4 + ki
                                    nc.tensor.matmul(
                                        fps[f // 2][:, f % 2, :],
                                        R(w_sb[:, kb, f * 128:(f + 1) * 128]),
                                        R(ht[:, ki, :]),
                                        start=False,
                                        stop=(kb == KB - 1),
                                    )
                        if kb4 == 0 and pending is not None:
                            emit_rope(pending)
                            pending = None
                    # clips immediately (free psum banks for the next tile)
                    cl5 = clp.tile([128, NF - 1, 512], F16, tag="cl", name="cl")
                    for f in range(NF):
                        pslice = fps[f // 2][:, f % 2, :]
                        if f == NF - 1:  # v: clip only
                            nc.vector.tensor_scalar(
                                v_t[:, s0:s0 + 512], pslice,
                                -CLIP, CLIP, Alu.max, Alu.min,
                            )
                        else:
                            nc.vector.tensor_scalar(
                                cl5[:, f, :], pslice,
                                -CLIP, CLIP, Alu.max, Alu.min,
                            )
                    vtr = v_t if s0 == S - 512 else None
                    pending = (b, s0, cso, cl5, cos_c, sin_c, vtr)
                emit_rope(pending)
                pending = None

            # ============ phase 2: attention + out_proj (per batch) ========
            with ExitStack() as actx:
                attnp = actx.enter_context(tc.tile_pool(name="attn", bufs=1))
                ptp = actx.enter_context(tc.tile_pool(name="pt", bufs=2))
                normp = actx.enter_context(tc.tile_pool(name="norm", bufs=2))
                owp = actx.enter_context(tc.tile_pool(name="ow", bufs=1))
                oevp = actx.enter_context(tc.tile_pool(name="oev", bufs=3))

                ow_sb = owp.tile([128, HPC, D], F16, tag="ow", name="ow")
                nc.sync.dma_start(
                    ow_sb[:], outw.ap().rearrange("(kb p) f -> p kb f", p=128)
                )

                for b in range(B):
                    at = attnp.tile(
                        [128, HPC, S], F16, tag="attn", name="attn"
                    )
                    for h in range(HPC):
                        qh_t = q_sb[:, b * HPC + h, :]
                        for qc in range(S // 1024):
                            qcnt = (b * HPC + h) * (S // 1024) + qc
                            ot = qcnt % 3
                            stl = [(ot + 1) % 3, (ot + 2) % 3]
                            q0 = qc * 1024
                            n_kb = min(SKB, (qc + 1) * 8) if causal else SKB
                            out_ps = psp.tile(
                                [128, 1024], F32, tag=f"p{ot}", name="outT"
                            )
                            l_ps = psp.tile([128, 1024], F32, tag="p3", name="l")

                            def seg_list(off):
                                # segments cut at psum bank bounds
                                segs = []
                                j = off
                                while j < 1024:
                                    nj = min(1024, (j // 512 + 1) * 512)
                                    segs.append((j, nj - j))
                                    j = nj
                                return segs

                            def qk_exp(kb):
                                qlo = max(q0, kb * 128) if causal else q0
                                off = qlo - q0
                                st = psp.tile(
                                    [128, 1024], F32, tag=f"p{stl[kb % 2]}",
                                    name="st",
                                )
                                for j, cw in seg_list(off):
                                    nc.tensor.matmul(
                                        st[:, j:j + cw],
                                        R(k_t[b][:, kb * 128:(kb + 1) * 128]),
                                        R(qh_t[:, q0 + j:q0 + j + cw]),
                                        start=True,
                                        stop=True,
                                    )
                                pt = ptp.tile(
                                    [128, 1024], F16, tag="pt", name="pt"
                                )
                                if causal and kb * 128 >= q0:
                                    # diagonal block: exp the 128 diag cols
                                    # first, mask them on gpsimd while the
                                    # rest exps, so PV waits less
                                    nc.scalar.activation(
                                        pt[:, off:off + 128],
                                        st[:, off:off + 128],
                                        Exp, bias=cbias[:], scale=ISQ,
                                    )
                                    nc.gpsimd.tensor_tensor(
                                        pt[:, off:off + 128],
                                        pt[:, off:off + 128],
                                        t01[:], Alu.mult,
                                    )
                                    if off + 128 < 1024:
                                        nc.scalar.activation(
                                            pt[:, off + 128:1024],
                                            st[:, off + 128:1024],
                                            Exp, bias=cbias[:], scale=ISQ,
                                        )
                                else:
                                    nc.scalar.activation(
                                        pt[:, off:1024], st[:, off:1024],
                                        Exp, bias=cbias[:], scale=ISQ,
                                    )
                                return pt, off

                            def pv_l(kb, pt, off):
                                first = kb == 0
                                last = kb == n_kb - 1
                                for j, cw in seg_list(off):
                                    nc.tensor.matmul(
                                        out_ps[:, j:j + cw],
                                        R(v_sb[b][:, kb, :]),
                                        R(pt[:, j:j + cw]),
                                        start=first,
                                        stop=last,
                                        skip_group_check=True,
                                    )
                                    nc.tensor.matmul(
                                        l_ps[:, j:j + cw],
                                        R(ones128[:]),
                                        R(pt[:, j:j + cw]),
                                        start=first,
                                        stop=last,
                                        skip_group_check=True,
                                    )

                            # software-pipeline: QK(kb+1) issues before
                            # PV/l(kb) so the PE never waits on exp(kb)
                            prev = None
                            for kb in range(n_kb):
                                cur = (kb, *qk_exp(kb))
                                if prev is not None:
                                    pv_l(*prev)
                                prev = cur
                            pv_l(*prev)
                            # normalize: 1/l (already broadcast across parts)
                            linv = normp.tile(
                                [128, 1024], F32, tag="linv", name="linv"
                            )
                            nc.vector.reciprocal_approx_fast(
                                linv[:], l_ps[:]
                            )
                            nc.vector.tensor_tensor(
                                at[:, h, q0:q0 + 1024], out_ps[:], linv[:],
                                Alu.mult,
                            )

                    # ---- out_proj partial for this batch ----
                    for mi in range(S // 128):
                        m = b * (S // 128) + mi
                        ml = mi * 128
                        for ofh in range(OFH):
                            of0 = ofh * OFW
                            po = psp.tile(
                                [128, 2, 512], F32,
                                tag=f"p{(m * OFH + ofh) % 3}",
                                name="po",
                            )
                            for kb in range(HPC):
                                for jj in range(2):
                                    nc.tensor.matmul(
                                        po[:, jj, :],
                                        R(at[:, kb, ml:ml + 128]),
                                        R(ow_sb[:, kb,
                                                of0 + jj * 512:of0 + (jj + 1) * 512]),
                                        start=(kb == 0),
                                        stop=(kb == HPC - 1),
                                        skip_group_check=True,
                                    )
                            # evict on DVE (2x for fp16 out), DMA from SBUF
                            oe = oevp.tile(
                                [128, 2, 512], F16, tag="oe", name="oe"
                            )
                            nc.vector.tensor_copy(oe[:], po[:])
                            nc.sync.dma_start(
                                out_d.ap()[m, ofh // 2, :,
                                           (ofh % 2) * 1024:(ofh % 2 + 1) * 1024],
                                oe[:],
                            )

        if reps > 1:
            rep_cm.__exit__(None, None, None)

    nc.compile()
    return nc


def rope_tables(position_ids, T):
    inv_freq = 1.0 / (
        ROPE_THETA ** (np.arange(0, DH, 2, dtype=np.float32) / DH)
    )
    freqs = (
        position_ids.astype(np.float32)[:, :, None] * inv_freq[None, None, :]
    )  # [B,S,64]
    emb = np.concatenate((freqs, freqs), axis=-1)  # [B,S,128]
    cos_t = np.ascontiguousarray(np.cos(emb).reshape(T, DH).T.astype(np.float32))
    sin_t = np.ascontiguousarray(np.sin(emb).reshape(T, DH).T.astype(np.float32))
    return cos_t, sin_t


def rot_matrix():
    """rotate_half as a matrix: rot(q) = R @ q for a [DH] head vector."""
    R = np.zeros((DH, DH), dtype=np.float32)
    half = DH // 2
    for d in range(half):
        R[d, d + half] = -1.0
        R[d + half, d] = 1.0
    return np.ascontiguousarray(R.T)  # lhsT for the PE


def tri01_mask():
    """[128,128] fp16 0/1 mask: zero where key row k > query col q."""
    ki, qj = np.meshgrid(np.arange(128), np.arange(128), indexing="ij")
    return (ki <= qj).astype(np.float16)


def make_host_inputs(hidden_states, position_ids, Wqkv_w, out_w, B, S, D):
    """Per-core input maps (host-side sharding / layout prep)."""
    T = B * S
    hid_t = np.ascontiguousarray(
        hidden_states.reshape(T, D).T.astype(np.float16)
    )
    cos_t, sin_t = rope_tables(position_ids, T)
    cos_t16 = cos_t.astype(np.float16)
    rot_t = rot_matrix().astype(np.float16)
    t01 = tri01_mask()
    idn = np.eye(128, dtype=np.float16)

    n_kv = D // 4  # KV_HEADS * HEAD_DIM
    in_maps = []
    for c in range(N_CORES):
        wq = Wqkv_w[c * HPC * DH:(c + 1) * HPC * DH]            # [512, D]
        wk = Wqkv_w[D + c * DH:D + (c + 1) * DH]                # [128, D]
        wv = Wqkv_w[D + n_kv + c * DH:D + n_kv + (c + 1) * DH]  # [128, D]
        wc = np.concatenate([wq, wk, wv], axis=0)               # [768, D]
        wc_t = np.ascontiguousarray(wc.T.astype(np.float16))    # [D, 768]
        ow_c = np.ascontiguousarray(
            out_w[:, c * HPC * DH:(c + 1) * HPC * DH].T.astype(np.float16)
        )  # [512, D]
        in_maps.append(
            {
                "hidden_t": hid_t,
                "wqkv_t": wc_t,
                "outw_t": ow_c,
                "cos_t": cos_t16,
                "sin_t": sin_t,
                "rot_t": rot_t,
                "trimask01": t01,
                "identity": idn,
            }
        )
    return in_maps


_PROGRAM_CACHE = {}


def _get_program(B, S, D, causal):
    key = (B, S, D, causal)
    if key not in _PROGRAM_CACHE:
        _PROGRAM_CACHE[key] = build_program(B, S, D, causal=causal)
    return _PROGRAM_CACHE[key]


def _detect_causal(attention_mask, B, S):
    causal = np.triu(
        np.full((S, S), np.finfo(np.float32).min, dtype=np.float32), 1
    )
    am = np.asarray(attention_mask)
    if am.shape == (B, 1, S, S):
        if np.array_equal(am, np.broadcast_to(causal[None, None], (B, 1, S, S))):
            return True
        if not am.any():
            return False
    raise ValueError(
        "kernel only supports the causal mask from setup_inputs() or an "
        "all-zero mask"
    )


def kernel(hidden_states, position_ids, attention_mask, Wqkv_w, out_w):
    hidden_states = np.asarray(hidden_states)
    position_ids = np.asarray(position_ids)
    Wqkv_w = np.asarray(Wqkv_w)
    out_w = np.asarray(out_w)

    B, S, D = hidden_states.shape
    causal = _detect_causal(attention_mask, B, S)
    nc = _get_program(B, S, D, causal)
    in_maps = make_host_inputs(
        hidden_states, position_ids, Wqkv_w, out_w, B, S, D
    )
    res = run_bass_kernel_spmd(nc, in_maps, list(range(N_CORES)))
    out = res.results[0]["out_partial"].astype(np.float32)
    for c in range(1, N_CORES):
        out += res.results[c]["out_partial"].astype(np.float32)
    # out is [MT, OH2, 128, OW2] tiled; reassemble to [B, S, D]
    mt, oh2, _, ow2 = out.shape
    out = out.transpose(0, 2, 1, 3).reshape(B, S, D)
    return out.astype(np.float32)



# revision 14
# speedup vs baseline: 1.0079x; 1.0079x over previous
"""Trainium2 Bass kernel for DBRX attention (B=2, S=2048, D=4096, 32 q-heads,
8 kv-heads GQA, causal, RoPE, fp32 reference), 8-way head-tensor-parallel.

Sharding: core c owns q-heads 4c..4c+3 and kv-head c (GQA groups stay
aligned). Each core computes its 512-dim slice of attention output, then a
full-token out_proj partial with its 512-row slice of out_w; the host sums
the 8 partials (the "all-reduce after out_proj" of the hint, done at gather
time).

Performance notes (v3):
  - ALL matmul operands are fp16 (FWL weight loads, no fp32r penalty).
  - Softmax row-sums moved OFF the PE: pt tiles accumulate across key
    blocks with DVE in-place fp16 adds, then one gpsimd
    partition_all_reduce per (head, q-chunk). Saves ~139k PE cycles and
    2 PSUM banks vs the ones-matmul scheme.
  - Causal mask folded into the QK PSUM group as a constant
    upper-triangular -2048 matmul against the identity: exp of a masked
    score underflows to exactly 0 in fp16. No exp split, no gpsimd mask.
  - Per-batch pipeline: QKV(b) -> attention(b) interleaved with
    out_proj(b): out_proj m-tiles are emitted as PE filler between
    attention heads so the Act engine's exp stream hides under out_proj
    matmuls instead of stalling PV.
  - out_proj evictions alternate DVE/Act; output stores go on the vector
    DMA queue (decoupled from the sync queue's hidden/weight loads).
  - softmax without a max pass: exp(S/sqrt(d) - C) with constant C; exact
    for any C (shift invariance); pt is fp16 so C=6 keeps the dominant
    weights in fp16 normal range.
"""

import math
import os
import sys

import numpy as np

for _p in ("/root/.axon_site/_ro/trn_rl_repo", "/opt/trn_rl_repo"):
    if os.path.isdir(_p) and _p not in sys.path:
        sys.path.append(_p)

import concourse.bass as bass
import concourse.tile as tile
from concourse import bacc, bass_isa, mybir
from concourse.bass_utils import run_bass_kernel_spmd

F32 = mybir.dt.float32
F16 = mybir.dt.float16


def R(ap):
    return ap

N_CORES = 8
DH = 128          # head dim
HPC = 4           # q heads per core
NF = HPC + 2      # qkv feature tiles of 128 per core (4 q + 1 k + 1 v)
CLIP = 8.0
ROPE_THETA = 500000.0
ISQ = 1.0 / math.sqrt(DH)
EXP_C = 6.0       # constant softmax shift (exact for any value; see header)
MASK_BIG = 2048.0  # causal mask additive constant (exp underflows to 0)


def build_program(B, S, D, causal=True, debug=False, reps=1):
    """Build the single-core Bass program (same program on all 8 cores)."""
    T = B * S                  # total tokens
    KB = D // 128              # contraction chunks for the projections
    SKB = S // 128             # k blocks per batch in attention
    QC = S // 1024             # query chunks per batch
    MPQ = 1024 // 128          # out_proj m-tiles per query chunk
    OFW = min(D, 1024)         # out-feature psum group width
    OFH = D // OFW
    OW2 = min(D, 2048)         # out eviction/DMA group width
    OH2 = D // OW2

    nc = bacc.Bacc(
        "TRN2",
        target_bir_lowering=False,
        debug=debug,
        num_devices=N_CORES,
    )

    hid = nc.dram_tensor("hidden_t", [D, T], F16, kind="ExternalInput")
    wqkv = nc.dram_tensor("wqkv_t", [D, NF * 128], F16, kind="ExternalInput")
    outw = nc.dram_tensor("outw_t", [HPC * DH, D], F16, kind="ExternalInput")
    cos_d = nc.dram_tensor("cos_t", [DH, T], F16, kind="ExternalInput")
    sin_d = nc.dram_tensor("sin_t", [DH, T], F32, kind="ExternalInput")
    rot_d = nc.dram_tensor("rot_t", [DH, DH], F16, kind="ExternalInput")
    tri_d = nc.dram_tensor("trimask", [128, 128], F16, kind="ExternalInput")
    idn_d = nc.dram_tensor("identity", [128, 128], F16, kind="ExternalInput")
    out_d = nc.dram_tensor("out_partial", [B * S // 128, OH2, 128, OW2], F16,
                           kind="ExternalOutput")

    Exp = mybir.ActivationFunctionType.Exp
    Alu = mybir.AluOpType

    from contextlib import ExitStack

    with ExitStack() as ctx:
        tc = ctx.enter_context(tile.TileContext(nc))
        PSUM = bass.MemorySpace.PSUM
        constp = ctx.enter_context(tc.tile_pool(name="const", bufs=1))
        # one PSUM pool, 4 tags x 2 banks, multiplexed across phases
        psp = ctx.enter_context(tc.tile_pool(name="psp", bufs=1, space=PSUM))

        # constants (loaded on the Act HWDGE queue so they don't delay the
        # sync queue's first hidden/weight loads)
        tri = constp.tile([128, 128], F16, tag="tri", name="tri")
        nc.scalar.dma_start(tri[:], tri_d.ap())
        idn = constp.tile([128, 128], F16, tag="idn", name="idn")
        nc.scalar.dma_start(idn[:], idn_d.ap())
        rott = constp.tile([DH, DH], F16, tag="rot", name="rot")
        nc.scalar.dma_start(rott[:], rot_d.ap())
        cbias = constp.tile([128, 1], F32, tag="cbias", name="cbias")
        nc.vector.memset(cbias[:], -EXP_C)

        if reps > 1:
            rep_cm = tc.For_i(0, reps, 1)
            rep_cm.__enter__()

        with ExitStack() as kvctx:
            wqp = kvctx.enter_context(tc.tile_pool(name="wq", bufs=1))
            owp = kvctx.enter_context(tc.tile_pool(name="ow", bufs=1))
            kvp = kvctx.enter_context(tc.tile_pool(name="kv", bufs=2))
            qresp = kvctx.enter_context(tc.tile_pool(name="qres", bufs=1))
            atp = kvctx.enter_context(tc.tile_pool(name="at", bufs=2))
            hidp = kvctx.enter_context(tc.tile_pool(name="hidp", bufs=2))
            csp = kvctx.enter_context(tc.tile_pool(name="cs", bufs=2))
            vtp = kvctx.enter_context(tc.tile_pool(name="vt", bufs=1))
            clp = kvctx.enter_context(tc.tile_pool(name="clp", bufs=2))
            workp = kvctx.enter_context(tc.tile_pool(name="work", bufs=2))
            ptp = kvctx.enter_context(tc.tile_pool(name="pt", bufs=3))
            ptap = kvctx.enter_context(tc.tile_pool(name="pta", bufs=2))
            lp = kvctx.enter_context(tc.tile_pool(name="l", bufs=1))
            oevp = kvctx.enter_context(tc.tile_pool(name="oev", bufs=3))

            # resident weights
            w_sb = wqp.tile([128, KB, NF * 128], F16, tag="w", name="w")
            ow_sb = owp.tile([128, HPC, D], F16, tag="ow", name="ow")
            nc.scalar.dma_start(
                ow_sb[:], outw.ap().rearrange("(kb p) f -> p kb f", p=128)
            )

            # FIFO of pending out_proj psum-groups (at_tile, m, ofh),
            # emitted as PE filler inside attention heads so the Act
            # engine's exp stream hides under out_proj matmuls. Survives
            # the batch boundary (at pool is double-buffered).
            fifo = []
            ev_cnt = [0]

            def emit_group(entry, tag, act_ok):
                at_t, m, ofh = entry
                ml = (m % (S // 128)) * 128
                of0 = ofh * OFW
                po = psp.tile([128, 2, 512], F32, tag=tag, name="po")
                for kb in range(HPC):
                    for jj in range(2):
                        nc.tensor.matmul(
                            po[:, jj, :],
                            R(at_t[:, kb, ml:ml + 128]),
                            R(ow_sb[:, kb,
                                    of0 + jj * 512:of0 + (jj + 1) * 512]),
                            start=(kb == 0),
                            stop=(kb == HPC - 1),
                            skip_group_check=True,
                        )
                oe = oevp.tile([128, 2, 512], F16, tag="oe", name="oe")
                if act_ok and ev_cnt[0] % 2 == 0:
                    nc.scalar.copy(oe[:], po[:])
                else:
                    nc.vector.tensor_copy(oe[:], po[:])
                ev_cnt[0] += 1
                nc.sync.dma_start(
                    out_d.ap()[m, ofh // 2, :,
                               (ofh % 2) * 1024:(ofh % 2 + 1) * 1024],
                    oe[:],
                )

            def drain(n, tag, act_ok=False):
                for _ in range(min(n, len(fifo))):
                    emit_group(fifo.pop(0), tag, act_ok)

            # ============ per-batch pipeline ============
            for b in range(B):
                k_t = kvp.tile([128, S], F16, tag="kt", name="kt")
                v_sb = kvp.tile([128, SKB, 128], F16, tag="v", name="v")
                q_sb = qresp.tile([128, HPC, S], F16, tag="q", name="q")
                at = atp.tile([128, HPC, S], F16, tag="attn", name="attn")

                # ---- phase 1: QKV + clip + RoPE for this batch ----
                v_t = vtp.tile([128, S], F16, tag="vt", name="vt")

                def emit_rope(pend):
                    """RoPE math for a previous 512-token tile (deferred so
                    its rot matmuls slot into the next tile's PE stream).
                    Also transposes this tile's V blocks to token-major so
                    the work is spread across phase 1 instead of bursting
                    at the phase boundary."""
                    s0, cso, cl5, cos_c, sin_c = pend
                    rps_t = psp.tile(
                        [128, 2, 512], F32, tag="p3", name="rotps"
                    )
                    for f in range(NF - 1):
                        cl = cl5[:, f, :]
                        rps = rps_t[:, f % 2, :]
                        nc.tensor.matmul(
                            rps, R(rott[:]), R(cl), start=True, stop=True
                        )
                        t1 = workp.tile([128, 512], F16, tag="t1", name="t1")
                        nc.vector.tensor_tensor(
                            t1[:], cl, cos_c[:, cso:cso + 512], Alu.mult
                        )
                        t2 = workp.tile([128, 512], F16, tag="t2", name="t2")
                        nc.vector.tensor_tensor(
                            t2[:], rps, sin_c[:, cso:cso + 512], Alu.mult
                        )
                        if f < HPC:
                            dest = q_sb[:, f, s0:s0 + 512]
                        else:
                            dest = k_t[:, s0:s0 + 512]
                        nc.vector.tensor_tensor(dest, t1[:], t2[:], Alu.add)
                    to0 = s0 // 128
                    tps = psp.tile([128, 4, 128], F16, tag="p3", name="vtps")
                    for ti in range(4):
                        nc.tensor.transpose(
                            R(tps[:, ti, :]),
                            R(v_t[:, (to0 + ti) * 128:(to0 + ti + 1) * 128]),
                            R(idn[:]),
                        )
                    nc.scalar.copy(v_sb[:, to0:to0 + 4, :], tps[:])

                pending = None
                for si in range(S // 512):
                    s0 = si * 512
                    t0 = b * S + s0

                    if t0 % 1024 == 0:
                        cos_c = csp.tile([DH, 1024], F16, tag="cos", name="cos")
                        nc.scalar.dma_start(
                            cos_c[:], cos_d.ap()[:, t0:t0 + 1024]
                        )
                        sin_c = csp.tile([DH, 1024], F32, tag="sin", name="sin")
                        nc.scalar.dma_start(
                            sin_c[:], sin_d.ap()[:, t0:t0 + 1024]
                        )
                    cso = t0 % 1024

                    fps = [
                        psp.tile([128, 2, 512], F32, tag=f"p{i}",
                                 name=f"qkvps{i}")
                        for i in range(NF // 2)
                    ]
                    for kb4 in range(KB // 4):
                        ht = hidp.tile([128, 4, 512], F16, tag="hid", name="hid")
                        nc.sync.dma_start(
                            ht[:],
                            hid.ap()[
                                kb4 * 512:(kb4 + 1) * 512, t0:t0 + 512
                            ].rearrange("(k p) c -> p k c", p=128),
                        )
                        if b == 0 and si == 0:
                            nc.sync.dma_start(
                                w_sb[:, kb4 * 4:(kb4 + 1) * 4, :],
                                wqkv.ap()[
                                    kb4 * 512:(kb4 + 1) * 512, :
                                ].rearrange("(kb p) f -> p kb f", p=128),
                            )
                        if kb4 < KB // 4 - 1:
                            for ki in range(4):
                                kb = kb4 * 4 + ki
                                for f in range(NF):
                                    nc.tensor.matmul(
                                        fps[f // 2][:, f % 2, :],
                                        R(w_sb[:, kb, f * 128:(f + 1) * 128]),
                                        R(ht[:, ki, :]),
                                        start=(kb == 0),
                                        stop=False,
                                    )
                        else:
                            # last contraction block: f-outer so each psum
                            # group completes (and can be clipped) early
                            for f in range(NF):
                                for ki in range(4):
                                    kb = kb4 * 4 + ki
                                    nc.tensor.matmul(
                                        fps[f // 2][:, f % 2, :],
                                        R(w_sb[:, kb, f * 128:(f + 1) * 128]),
                                        R(ht[:, ki, :]),
                                        start=False,
                                        stop=(kb == KB - 1),
                                    )
                        if kb4 == 0 and pending is not None:
                            emit_rope(pending)
                            pending = None
                    # clips immediately (free psum banks for the next tile)
                    cl5 = clp.tile([128, NF - 1, 512], F16, tag="cl", name="cl")
                    for f in range(NF):
                        pslice = fps[f // 2][:, f % 2, :]
                        if f == NF - 1:  # v: clip only
                            nc.vector.tensor_scalar(
                                v_t[:, s0:s0 + 512], pslice,
                                -CLIP, CLIP, Alu.max, Alu.min,
                            )
                        else:
                            nc.vector.tensor_scalar(
                                cl5[:, f, :], pslice,
                                -CLIP, CLIP, Alu.max, Alu.min,
                            )
                    pending = (s0, cso, cl5, cos_c, sin_c)
                emit_rope(pending)
                pending = None

                # ---- phase 2: attention interleaved with out_proj ----
                hcnt = 0
                for qc in range(QC):
                    q0 = qc * 1024
                    n_kb = min(SKB, (qc + 1) * 8) if causal else SKB
                    for h in range(HPC):
                        qh_t = q_sb[:, h, :]
                        out_ps = psp.tile(
                            [128, 1024], F32, tag=f"p{2 + hcnt % 2}",
                            name="outT",
                        )
                        ftag = f"p{2 + (hcnt + 1) % 2}"
                        hcnt += 1
                        pta = ptap.tile([128, 1024], F16, tag="pta", name="pta")

                        def seg_list(off):
                            # segments cut at psum bank bounds
                            segs = []
                            j = off
                            while j < 1024:
                                nj = min(1024, (j // 512 + 1) * 512)
                                segs.append((j, nj - j))
                                j = nj
                            return segs

                        def qk_exp(kb):
                            qlo = max(q0, kb * 128) if causal else q0
                            off = qlo - q0
                            st = psp.tile(
                                [128, 1024], F32, tag=f"p{kb % 2}", name="st",
                            )
                            diag = causal and kb * 128 >= q0
                            for j, cw in seg_list(off):
                                nc.tensor.matmul(
                                    st[:, j:j + cw],
                                    R(k_t[:, kb * 128:(kb + 1) * 128]),
                                    R(qh_t[:, q0 + j:q0 + j + cw]),
                                    start=True,
                                    stop=not (diag and j == off),
                                )
                            if diag:
                                # fold the causal mask into the psum group:
                                # st[:, off:off+128] += tri(-2048)
                                nc.tensor.matmul(
                                    st[:, off:off + 128],
                                    R(tri[:]),
                                    R(idn[:]),
                                    start=False,
                                    stop=True,
                                    skip_group_check=True,
                                )
                            pt = ptp.tile([128, 1024], F16, tag="pt", name="pt")
                            nc.scalar.activation(
                                pt[:, off:1024], st[:, off:1024],
                                Exp, bias=cbias[:], scale=ISQ,
                            )
                            # row-sum accumulate on DVE (fp16 2x mode)
                            if kb == 0:
                                nc.vector.tensor_copy(pta[:], pt[:])
                            else:
                                nc.vector.tensor_tensor(
                                    pta[:, off:1024], pta[:, off:1024],
                                    pt[:, off:1024], Alu.add,
                                )
                            return pt, off

                        def pv(kb, pt, off):
                            first = kb == 0
                            last = kb == n_kb - 1
                            for j, cw in seg_list(off):
                                nc.tensor.matmul(
                                    out_ps[:, j:j + cw],
                                    R(v_sb[:, kb, :]),
                                    R(pt[:, j:j + cw]),
                                    start=first,
                                    stop=last,
                                    skip_group_check=True,
                                )

                        # software-pipeline: QK(kb+1) issues before PV(kb)
                        # so the PE never waits on exp(kb). out_proj groups
                        # are spliced in as PE filler: the lead-in pair
                        # covers the previous head's l-chain latency (WAR
                        # on out_ps), the mid-head ones let Act catch up.
                        prev = None
                        for kb in range(n_kb):
                            cur = (kb, *qk_exp(kb))
                            if prev is not None:
                                if prev[0] == 0:
                                    drain(1, ftag)
                                pv(*prev)
                                if prev[0] in (0, 3, 7, 11) and kb < n_kb - 1:
                                    drain(1, ftag)
                            prev = cur
                        pv(*prev)
                        # softmax denominator: partition reduce on gpsimd
                        # (broadcast across partitions), then 1/l on DVE
                        l_f = lp.tile([128, 1024], F32, tag="l", name="l")
                        nc.gpsimd.partition_all_reduce(
                            l_f[:], pta[:], channels=128,
                            reduce_op=bass_isa.ReduceOp.add,
                        )
                        linv = lp.tile([128, 1024], F32, tag="linv",
                                       name="linv")
                        nc.vector.reciprocal_approx_fast(linv[:], l_f[:])
                        nc.vector.tensor_tensor(
                            at[:, h, q0:q0 + 1024], out_ps[:], linv[:],
                            Alu.mult,
                        )
                    for i in range(MPQ):
                        m = b * (S // 128) + qc * MPQ + i
                        for ofh in range(OFH):
                            fifo.append((at, m, ofh))
                # flush: emit the backlog (Act is free to take half the
                # evictions here). Cross-batch carry-over is NOT worth it:
                # the tile framework's semaphore encoding over-syncs stale
                # at-tile reads against the whole DVE prefix.
                fcnt = 0
                while fifo:
                    emit_group(fifo.pop(0), f"p{2 + fcnt % 2}", act_ok=True)
                    fcnt += 1

        if reps > 1:
            rep_cm.__exit__(None, None, None)

    nc.compile()
    return nc


def rope_tables(position_ids, T):
    inv_freq = 1.0 / (
        ROPE_THETA ** (np.arange(0, DH, 2, dtype=np.float32) / DH)
    )
    freqs = (
        position_ids.astype(np.float32)[:, :, None] * inv_freq[None, None, :]
    )  # [B,S,64]
    emb = np.concatenate((freqs, freqs), axis=-1)  # [B,S,128]
    cos_t = np.ascontiguousarray(np.cos(emb).reshape(T, DH).T.astype(np.float32))
    sin_t = np.ascontiguousarray(np.sin(emb).reshape(T, DH).T.astype(np.float32))
    return cos_t, sin_t


def rot_matrix():
    """rotate_half as a matrix: rot(q) = R @ q for a [DH] head vector."""
    R = np.zeros((DH, DH), dtype=np.float32)
    half = DH // 2
    for d in range(half):
        R[d, d + half] = -1.0
        R[d + half, d] = 1.0
    return np.ascontiguousarray(R.T)  # lhsT for the PE


def tri_mask():
    """[128,128] fp16 lhsT for the causal mask matmul: out[k, q] +=
    tri[q, k] must be -MASK_BIG where key k > query q (local indices)."""
    qi, ki = np.meshgrid(np.arange(128), np.arange(128), indexing="ij")
    return ((ki > qi) * -MASK_BIG).astype(np.float16)


def make_host_inputs(hidden_states, position_ids, Wqkv_w, out_w, B, S, D):
    """Per-core input maps (host-side sharding / layout prep)."""
    T = B * S
    hid_t = np.ascontiguousarray(
        hidden_states.reshape(T, D).T.astype(np.float16)
    )
    cos_t, sin_t = rope_tables(position_ids, T)
    cos_t16 = cos_t.astype(np.float16)
    rot_t = rot_matrix().astype(np.float16)
    tri = tri_mask()
    idn = np.eye(128, dtype=np.float16)

    n_kv = D // 4  # KV_HEADS * HEAD_DIM
    in_maps = []
    for c in range(N_CORES):
        wq = Wqkv_w[c * HPC * DH:(c + 1) * HPC * DH]            # [512, D]
        wk = Wqkv_w[D + c * DH:D + (c + 1) * DH]                # [128, D]
        wv = Wqkv_w[D + n_kv + c * DH:D + n_kv + (c + 1) * DH]  # [128, D]
        wc = np.concatenate([wq, wk, wv], axis=0)               # [768, D]
        wc_t = np.ascontiguousarray(wc.T.astype(np.float16))    # [D, 768]
        ow_c = np.ascontiguousarray(
            out_w[:, c * HPC * DH:(c + 1) * HPC * DH].T.astype(np.float16)
        )  # [512, D]
        in_maps.append(
            {
                "hidden_t": hid_t,
                "wqkv_t": wc_t,
                "outw_t": ow_c,
                "cos_t": cos_t16,
                "sin_t": sin_t,
                "rot_t": rot_t,
                "trimask": tri,
                "identity": idn,
            }
        )
    return in_maps


_PROGRAM_CACHE = {}


def _get_program(B, S, D, causal):
    key = (B, S, D, causal)
    if key not in _PROGRAM_CACHE:
        _PROGRAM_CACHE[key] = build_program(B, S, D, causal=causal)
    return _PROGRAM_CACHE[key]


def _detect_causal(attention_mask, B, S):
    causal = np.triu(
        np.full((S, S), np.finfo(np.float32).min, dtype=np.float32), 1
    )
    am = np.asarray(attention_mask)
    if am.shape == (B, 1, S, S):
        if np.array_equal(am, np.broadcast_to(causal[None, None], (B, 1, S, S))):
            return True
        if not am.any():
            return False
    raise ValueError(
        "kernel only supports the causal mask from setup_inputs() or an "
        "all-zero mask"
    )


def kernel(hidden_states, position_ids, attention_mask, Wqkv_w, out_w):
    hidden_states = np.asarray(hidden_states)
    position_ids = np.asarray(position_ids)
    Wqkv_w = np.asarray(Wqkv_w)
    out_w = np.asarray(out_w)

    B, S, D = hidden_states.shape
    causal = _detect_causal(attention_mask, B, S)
    nc = _get_program(B, S, D, causal)
    in_maps = make_host_inputs(
        hidden_states, position_ids, Wqkv_w, out_w, B, S, D
    )
    res = run_bass_kernel_spmd(nc, in_maps, list(range(N_CORES)))
    out = res.results[0]["out_partial"].astype(np.float32)
    for c in range(1, N_CORES):
        out += res.results[c]["out_partial"].astype(np.float32)
    # out is [MT, OH2, 128, OW2] tiled; reassemble to [B, S, D]
    mt, oh2, _, ow2 = out.shape
    out = out.transpose(0, 2, 1, 3).reshape(B, S, D)
    return out.astype(np.float32)


# revision 25
# speedup vs baseline: 1.1388x; 1.1299x over previous
"""Trainium2 Bass kernel for DBRX attention (B=2, S=2048, D=4096, 32 q-heads,
8 kv-heads GQA, causal, RoPE, fp32 reference), 8-way head-tensor-parallel.

Sharding: core c owns q-heads 4c..4c+3 and kv-head c (GQA groups stay
aligned). Each core computes its 512-dim slice of attention output, then a
full-token out_proj partial with its 512-row slice of out_w; the host sums
the 8 partials (the "all-reduce after out_proj" of the hint, done at gather
time).

Performance notes (v3):
  - ALL matmul operands are fp16 (FWL weight loads, no fp32r penalty).
  - Softmax row-sums moved OFF the PE: pt tiles accumulate across key
    blocks with DVE in-place fp16 adds, then one gpsimd
    partition_all_reduce per (head, q-chunk). Saves ~139k PE cycles and
    2 PSUM banks vs the ones-matmul scheme.
  - Causal mask folded into the QK PSUM group as a constant
    upper-triangular -2048 matmul against the identity: exp of a masked
    score underflows to exactly 0 in fp16. No exp split, no gpsimd mask.
  - Per-batch pipeline: QKV(b) -> attention(b) interleaved with
    out_proj(b): out_proj m-tiles are emitted as PE filler between
    attention heads so the Act engine's exp stream hides under out_proj
    matmuls instead of stalling PV.
  - out_proj evictions alternate DVE/Act; output stores go on the vector
    DMA queue (decoupled from the sync queue's hidden/weight loads).
  - softmax without a max pass: exp(S/sqrt(d) - C) with constant C; exact
    for any C (shift invariance); pt is fp16 so C=6 keeps the dominant
    weights in fp16 normal range.
"""

import math
import os
import sys

import numpy as np

for _p in ("/root/.axon_site/_ro/trn_rl_repo", "/opt/trn_rl_repo"):
    if os.path.isdir(_p) and _p not in sys.path:
        sys.path.append(_p)

import concourse.bass as bass
import concourse.tile as tile
from concourse import bacc, bass_isa, mybir
from concourse.bass_utils import run_bass_kernel_spmd

F32 = mybir.dt.float32
F16 = mybir.dt.float16


def R(ap):
    return ap

N_CORES = 8
DH = 128          # head dim
HPC = 4           # q heads per core
NF = HPC + 2      # qkv feature tiles of 128 per core (4 q + 1 k + 1 v)
CLIP = 8.0
ROPE_THETA = 500000.0
ISQ = 1.0 / math.sqrt(DH)
EXP_C = 6.0       # constant softmax shift (exact for any value; see header)
MASK_BIG = 2048.0  # causal mask additive constant (exp underflows to 0)


def build_program(B, S, D, causal=True, debug=False, reps=1, phase="all"):
    """Build the single-core Bass program (same program on all 8 cores).

    phase: "all" for the real kernel; "qkv"/"outproj"/"dma" build isolated
    micro-bench variants (timing only, outputs are garbage).
    """
    do_qkv = phase in ("all", "qkv")
    do_attn = phase in ("all", "attn")
    do_outproj = phase in ("all", "outproj")
    T = B * S                  # total tokens
    KB = D // 128              # contraction chunks for the projections
    SKB = S // 128             # k blocks per batch in attention
    QC = S // 1024             # query chunks per batch
    MPQ = 1024 // 128          # out_proj m-tiles per query chunk
    OFW = min(D, 1024)         # out-feature psum group width
    OFH = D // OFW
    OW2 = min(D, 2048)         # out eviction/DMA group width
    OH2 = D // OW2

    nc = bacc.Bacc(
        "TRN2",
        target_bir_lowering=False,
        debug=debug,
        num_devices=N_CORES,
    )

    hid = nc.dram_tensor("hidden_t", [D, T], F16, kind="ExternalInput")
    wqkv = nc.dram_tensor("wqkv_t", [D, NF * 128], F16, kind="ExternalInput")
    outw = nc.dram_tensor("outw_t", [HPC * DH, D], F16, kind="ExternalInput")
    cos_d = nc.dram_tensor("cos_t", [DH, T], F16, kind="ExternalInput")
    sin_d = nc.dram_tensor("sin_t", [DH, T], F32, kind="ExternalInput")
    rot_d = nc.dram_tensor("rot_t", [DH, DH], F16, kind="ExternalInput")
    tri_d = nc.dram_tensor("trimask", [128, 128], F16, kind="ExternalInput")
    idn_d = nc.dram_tensor("identity", [128, 128], F16, kind="ExternalInput")
    out_d = nc.dram_tensor("out_partial", [B * S // 128, OH2, 128, OW2], F16,
                           kind="ExternalOutput")

    Exp = mybir.ActivationFunctionType.Exp
    Alu = mybir.AluOpType

    from contextlib import ExitStack

    with ExitStack() as ctx:
        tc = ctx.enter_context(tile.TileContext(nc))
        PSUM = bass.MemorySpace.PSUM
        constp = ctx.enter_context(tc.tile_pool(name="const", bufs=1))
        # one PSUM pool, 4 tags x 2 banks, multiplexed across phases
        psp = ctx.enter_context(tc.tile_pool(name="psp", bufs=1, space=PSUM))

        # constants (loaded on the Act HWDGE queue so they don't delay the
        # sync queue's first hidden/weight loads)
        tri = constp.tile([128, 128], F16, tag="tri", name="tri")
        nc.scalar.dma_start(tri[:], tri_d.ap())
        idn = constp.tile([128, 128], F16, tag="idn", name="idn")
        nc.scalar.dma_start(idn[:], idn_d.ap())
        rott = constp.tile([DH, DH], F16, tag="rot", name="rot")
        nc.scalar.dma_start(rott[:], rot_d.ap())
        cbias = constp.tile([128, 1], F32, tag="cbias", name="cbias")
        nc.vector.memset(cbias[:], -EXP_C)
        ones128 = constp.tile([128, 128], F16, tag="ones", name="ones")
        nc.vector.memset(ones128[:], 1.0)

        if reps > 1:
            rep_cm = tc.For_i(0, reps, 1)
            rep_cm.__enter__()

        with ExitStack() as kvctx:
            wqp = kvctx.enter_context(tc.tile_pool(name="wq", bufs=1))
            owp = kvctx.enter_context(tc.tile_pool(name="ow", bufs=1))
            kvp = kvctx.enter_context(tc.tile_pool(name="kv", bufs=2))
            qresp = kvctx.enter_context(tc.tile_pool(name="qres", bufs=1))
            atp = kvctx.enter_context(tc.tile_pool(name="at", bufs=2))
            hidp = kvctx.enter_context(tc.tile_pool(name="hidp", bufs=2))
            csp = kvctx.enter_context(tc.tile_pool(name="cs", bufs=2))
            vtp = kvctx.enter_context(tc.tile_pool(name="vt", bufs=1))
            clp = kvctx.enter_context(tc.tile_pool(name="clp", bufs=2))
            workp = kvctx.enter_context(tc.tile_pool(name="work", bufs=2))
            ptp = kvctx.enter_context(tc.tile_pool(name="pt", bufs=3))
            ptap = kvctx.enter_context(tc.tile_pool(name="pta", bufs=2))
            lp = kvctx.enter_context(tc.tile_pool(name="l", bufs=1))
            oevp = kvctx.enter_context(tc.tile_pool(name="oev", bufs=3))

            # resident weights
            w_sb = wqp.tile([128, KB, NF * 128], F16, tag="w", name="w")
            ow_sb = owp.tile([128, HPC, D], F16, tag="ow", name="ow")
            nc.scalar.dma_start(
                ow_sb[:], outw.ap().rearrange("(kb p) f -> p kb f", p=128)
            )

            # FIFO of pending out_proj psum-groups (at_tile, m, ofh),
            # emitted as PE filler inside attention heads so the Act
            # engine's exp stream hides under out_proj matmuls. Survives
            # the batch boundary (at pool is double-buffered).
            fifo = []
            ev_cnt = [0]

            def emit_group(entry, tag, act_ok):
                at_t, m, ofh = entry
                ml = (m % (S // 128)) * 128
                of0 = ofh * OFW
                po = psp.tile([128, 2, 512], F32, tag=tag, name="po")
                for kb in range(HPC):
                    for jj in range(2):
                        nc.tensor.matmul(
                            po[:, jj, :],
                            R(at_t[:, kb, ml:ml + 128]),
                            R(ow_sb[:, kb,
                                    of0 + jj * 512:of0 + (jj + 1) * 512]),
                            start=(kb == 0),
                            stop=(kb == HPC - 1),
                            skip_group_check=True,
                        )
                oe = oevp.tile([128, 2, 512], F16, tag="oe", name="oe")
                if act_ok and ev_cnt[0] % 2 == 0:
                    nc.scalar.copy(oe[:], po[:])
                else:
                    nc.vector.tensor_copy(oe[:], po[:])
                ev_cnt[0] += 1
                nc.sync.dma_start(
                    out_d.ap()[m, ofh // 2, :,
                               (ofh % 2) * 1024:(ofh % 2 + 1) * 1024],
                    oe[:],
                )

            def drain(n, tag, act_ok=False):
                for _ in range(min(n, len(fifo))):
                    emit_group(fifo.pop(0), tag, act_ok)

            # ============ per-batch pipeline ============
            for b in range(B):
                k_t = kvp.tile([128, S], F16, tag="kt", name="kt")
                v_sb = kvp.tile([128, SKB, 128], F16, tag="v", name="v")
                q_sb = qresp.tile([128, HPC, S], F16, tag="q", name="q")
                at = atp.tile([128, HPC, S], F16, tag="attn", name="attn")

                # ---- phase 1: QKV + clip + RoPE for this batch ----
                v_t = vtp.tile([128, S], F16, tag="vt", name="vt")

                if phase == "attn":
                    nc.vector.memset(k_t[:], 0.01)
                    nc.vector.memset(v_sb[:], 0.01)
                    nc.vector.memset(q_sb[:], 0.01)
                if phase == "dma":
                    for si in range(S // 512):
                        t0 = b * S + si * 512
                        for kb4 in range(KB // 4):
                            ht = hidp.tile([128, 4, 512], F16, tag="hid",
                                           name="hid")
                            nc.sync.dma_start(
                                ht[:],
                                hid.ap()[
                                    kb4 * 512:(kb4 + 1) * 512, t0:t0 + 512
                                ].rearrange("(k p) c -> p k c", p=128),
                            )
                        nc.sync.dma_start(
                            out_d.ap()[b * (S // 128) + si, 0, :, :],
                            ht[:, 0:4, :],
                        )
                    continue
                if phase == "outproj":
                    nc.vector.memset(at[:], 0.00391)
                    for mi in range(S // 128):
                        m = b * (S // 128) + mi
                        for ofh in range(OFH):
                            emit_group((at, m, ofh),
                                       f"p{(mi * OFH + ofh) % 2}",
                                       act_ok=True)
                    continue

                def emit_rope(pend):
                    """RoPE math for a previous 512-token tile (deferred so
                    its rot matmuls slot into the next tile's PE stream).
                    Also transposes this tile's V blocks to token-major so
                    the work is spread across phase 1 instead of bursting
                    at the phase boundary."""
                    s0, cso, cl5, cos_c, sin_c = pend
                    rps_t = psp.tile(
                        [128, 2, 512], F32, tag="p3", name="rotps"
                    )
                    for f in range(NF - 1):
                        cl = cl5[:, f, :]
                        rps = rps_t[:, f % 2, :]
                        nc.tensor.matmul(
                            rps, R(rott[:]), R(cl), start=True, stop=True
                        )
                        t1 = workp.tile([128, 512], F16, tag="t1", name="t1")
                        nc.vector.tensor_tensor(
                            t1[:], cl, cos_c[:, cso:cso + 512], Alu.mult
                        )
                        t2 = workp.tile([128, 512], F16, tag="t2", name="t2")
                        nc.vector.tensor_tensor(
                            t2[:], rps, sin_c[:, cso:cso + 512], Alu.mult
                        )
                        if f < HPC:
                            dest = q_sb[:, f, s0:s0 + 512]
                        else:
                            dest = k_t[:, s0:s0 + 512]
                        nc.vector.tensor_tensor(dest, t1[:], t2[:], Alu.add)
                    to0 = s0 // 128
                    tps = psp.tile([128, 4, 128], F16, tag="p3", name="vtps")
                    for ti in range(4):
                        nc.tensor.transpose(
                            R(tps[:, ti, :]),
                            R(v_t[:, (to0 + ti) * 128:(to0 + ti + 1) * 128]),
                            R(idn[:]),
                        )
                    nc.scalar.copy(v_sb[:, to0:to0 + 4, :], tps[:])

                pending = None
                for si in range(S // 512 if do_qkv else 0):
                    s0 = si * 512
                    t0 = b * S + s0

                    if t0 % 1024 == 0:
                        cos_c = csp.tile([DH, 1024], F16, tag="cos", name="cos")
                        nc.scalar.dma_start(
                            cos_c[:], cos_d.ap()[:, t0:t0 + 1024]
                        )
                        sin_c = csp.tile([DH, 1024], F32, tag="sin", name="sin")
                        nc.scalar.dma_start(
                            sin_c[:], sin_d.ap()[:, t0:t0 + 1024]
                        )
                    cso = t0 % 1024

                    fps = [
                        psp.tile([128, 2, 512], F32, tag=f"p{i}",
                                 name=f"qkvps{i}")
                        for i in range(NF // 2)
                    ]
                    for kb4 in range(KB // 4):
                        ht = hidp.tile([128, 4, 512], F16, tag="hid", name="hid")
                        nc.sync.dma_start(
                            ht[:],
                            hid.ap()[
                                kb4 * 512:(kb4 + 1) * 512, t0:t0 + 512
                            ].rearrange("(k p) c -> p k c", p=128),
                        )
                        if b == 0 and si == 0:
                            nc.sync.dma_start(
                                w_sb[:, kb4 * 4:(kb4 + 1) * 4, :],
                                wqkv.ap()[
                                    kb4 * 512:(kb4 + 1) * 512, :
                                ].rearrange("(kb p) f -> p kb f", p=128),
                            )
                        if kb4 < KB // 4 - 1:
                            for ki in range(4):
                                kb = kb4 * 4 + ki
                                for f in range(NF):
                                    nc.tensor.matmul(
                                        fps[f // 2][:, f % 2, :],
                                        R(w_sb[:, kb, f * 128:(f + 1) * 128]),
                                        R(ht[:, ki, :]),
                                        start=(kb == 0),
                                        stop=False,
                                    )
                        else:
                            # last contraction block: f-outer so each psum
                            # group completes (and can be clipped) early
                            for f in range(NF):
                                for ki in range(4):
                                    kb = kb4 * 4 + ki
                                    nc.tensor.matmul(
                                        fps[f // 2][:, f % 2, :],
                                        R(w_sb[:, kb, f * 128:(f + 1) * 128]),
                                        R(ht[:, ki, :]),
                                        start=False,
                                        stop=(kb == KB - 1),
                                    )
                        if kb4 == 0 and pending is not None:
                            emit_rope(pending)
                            pending = None
                    # clips immediately (free psum banks for the next tile)
                    cl5 = clp.tile([128, NF - 1, 512], F16, tag="cl", name="cl")
                    for f in range(NF):
                        pslice = fps[f // 2][:, f % 2, :]
                        if f == NF - 1:  # v: clip only
                            nc.vector.tensor_scalar(
                                v_t[:, s0:s0 + 512], pslice,
                                -CLIP, CLIP, Alu.max, Alu.min,
                            )
                        else:
                            nc.vector.tensor_scalar(
                                cl5[:, f, :], pslice,
                                -CLIP, CLIP, Alu.max, Alu.min,
                            )
                    pending = (s0, cso, cl5, cos_c, sin_c)
                if pending is not None:
                    emit_rope(pending)
                pending = None

                if not do_attn:
                    nc.sync.dma_start(
                        out_d.ap()[b * (S // 128), 0, :, :], k_t[:, 0:2048]
                    )
                    continue

                # ---- phase 2: attention interleaved with out_proj ----
                hcnt = 0
                for qc in range(QC):
                    q0 = qc * 1024
                    n_kb = min(SKB, (qc + 1) * 8) if causal else SKB
                    for h in range(HPC):
                        qh_t = q_sb[:, h, :]
                        out_ps = psp.tile(
                            [128, 1024], F32, tag=f"p{2 + hcnt % 2}",
                            name="outT",
                        )
                        ftag = f"p{2 + (hcnt + 1) % 2}"
                        hcnt += 1
                        pta = ptap.tile([128, 1024], F16, tag="pta", name="pta")

                        def seg_list(off):
                            # segments cut at psum bank bounds
                            segs = []
                            j = off
                            while j < 1024:
                                nj = min(1024, (j // 512 + 1) * 512)
                                segs.append((j, nj - j))
                                j = nj
                            return segs

                        def qk_exp(kb):
                            qlo = max(q0, kb * 128) if causal else q0
                            off = qlo - q0
                            st = psp.tile(
                                [128, 1024], F32, tag=f"p{kb % 2}", name="st",
                            )
                            diag = causal and kb * 128 >= q0
                            for j, cw in seg_list(off):
                                nc.tensor.matmul(
                                    st[:, j:j + cw],
                                    R(k_t[:, kb * 128:(kb + 1) * 128]),
                                    R(qh_t[:, q0 + j:q0 + j + cw]),
                                    start=True,
                                    stop=not (diag and j == off),
                                )
                            if diag:
                                # fold the causal mask into the psum group:
                                # st[:, off:off+128] += tri(-2048)
                                nc.tensor.matmul(
                                    st[:, off:off + 128],
                                    R(tri[:]),
                                    R(idn[:]),
                                    start=False,
                                    stop=True,
                                    skip_group_check=True,
                                )
                            pt = ptp.tile([128, 1024], F16, tag="pt", name="pt")
                            nc.scalar.activation(
                                pt[:, off:1024], st[:, off:1024],
                                Exp, bias=cbias[:], scale=ISQ,
                            )
                            # row-sum accumulate on DVE (fp16 2x mode)
                            if kb == 0:
                                nc.vector.tensor_copy(pta[:], pt[:])
                            else:
                                nc.vector.tensor_tensor(
                                    pta[:, off:1024], pta[:, off:1024],
                                    pt[:, off:1024], Alu.add,
                                )
                            return pt, off

                        def pv(kb, pt, off):
                            first = kb == 0
                            last = kb == n_kb - 1
                            for j, cw in seg_list(off):
                                nc.tensor.matmul(
                                    out_ps[:, j:j + cw],
                                    R(v_sb[:, kb, :]),
                                    R(pt[:, j:j + cw]),
                                    start=first,
                                    stop=last,
                                    skip_group_check=True,
                                )

                        # software-pipeline: QK(kb+1) issues before PV(kb)
                        # so the PE never waits on exp(kb). out_proj groups
                        # are spliced in as PE filler: the lead-in pair
                        # covers the previous head's l-chain latency (WAR
                        # on out_ps), the mid-head ones let Act catch up.
                        prev = None
                        for kb in range(n_kb):
                            cur = (kb, *qk_exp(kb))
                            if prev is not None:
                                if prev[0] == 0:
                                    drain(1, ftag)
                                pv(*prev)
                                if prev[0] in (0, 3, 7, 11) and kb < n_kb - 1:
                                    drain(1, ftag)
                            prev = cur
                        pv(*prev)
                        # softmax denominator: one ones-matmul reduces pta
                        # over partitions (result broadcast across all 128
                        # psum partitions), then 1/l on DVE. Runs on the
                        # filler tag between out_proj drains.
                        l_ps = psp.tile([128, 1024], F32, tag=ftag, name="lps")
                        for j in (0, 512):
                            nc.tensor.matmul(
                                l_ps[:, j:j + 512], R(ones128[:]),
                                R(pta[:, j:j + 512]),
                                start=True, stop=True, skip_group_check=True,
                            )
                        linv = lp.tile([128, 1024], F32, tag="linv",
                                       name="linv")
                        nc.vector.reciprocal_approx_fast(linv[:], l_ps[:])
                        nc.vector.tensor_tensor(
                            at[:, h, q0:q0 + 1024], out_ps[:], linv[:],
                            Alu.mult,
                        )
                    for i in range(MPQ):
                        m = b * (S // 128) + qc * MPQ + i
                        for ofh in range(OFH):
                            fifo.append((at, m, ofh))
                # flush: emit the backlog (Act is free to take half the
                # evictions here). Cross-batch carry-over is NOT worth it:
                # the tile framework's semaphore encoding over-syncs stale
                # at-tile reads against the whole DVE prefix.
                fcnt = 0
                while fifo:
                    emit_group(fifo.pop(0), f"p{2 + fcnt % 2}", act_ok=True)
                    fcnt += 1

        if reps > 1:
            rep_cm.__exit__(None, None, None)

    nc.compile()
    return nc


def rope_tables(position_ids, T):
    inv_freq = 1.0 / (
        ROPE_THETA ** (np.arange(0, DH, 2, dtype=np.float32) / DH)
    )
    freqs = (
        position_ids.astype(np.float32)[:, :, None] * inv_freq[None, None, :]
    )  # [B,S,64]
    emb = np.concatenate((freqs, freqs), axis=-1)  # [B,S,128]
    cos_t = np.ascontiguousarray(np.cos(emb).reshape(T, DH).T.astype(np.float32))
    sin_t = np.ascontiguousarray(np.sin(emb).reshape(T, DH).T.astype(np.float32))
    return cos_t, sin_t


def rot_matrix():
    """rotate_half as a matrix: rot(q) = R @ q for a [DH] head vector."""
    R = np.zeros((DH, DH), dtype=np.float32)
    half = DH // 2
    for d in range(half):
        R[d, d + half] = -1.0
        R[d + half, d] = 1.0
    return np.ascontiguousarray(R.T)  # lhsT for the PE


def tri_mask():
    """[128,128] fp16 lhsT for the causal mask matmul: out[k, q] +=
    tri[q, k] must be -MASK_BIG where key k > query q (local indices)."""
    qi, ki = np.meshgrid(np.arange(128), np.arange(128), indexing="ij")
    return ((ki > qi) * -MASK_BIG).astype(np.float16)


def make_host_inputs(hidden_states, position_ids, Wqkv_w, out_w, B, S, D):
    """Per-core input maps (host-side sharding / layout prep)."""
    T = B * S
    hid_t = np.ascontiguousarray(
        hidden_states.reshape(T, D).T.astype(np.float16)
    )
    cos_t, sin_t = rope_tables(position_ids, T)
    cos_t16 = cos_t.astype(np.float16)
    rot_t = rot_matrix().astype(np.float16)
    tri = tri_mask()
    idn = np.eye(128, dtype=np.float16)

    n_kv = D // 4  # KV_HEADS * HEAD_DIM
    in_maps = []
    for c in range(N_CORES):
        wq = Wqkv_w[c * HPC * DH:(c + 1) * HPC * DH]            # [512, D]
        wk = Wqkv_w[D + c * DH:D + (c + 1) * DH]                # [128, D]
        wv = Wqkv_w[D + n_kv + c * DH:D + n_kv + (c + 1) * DH]  # [128, D]
        wc = np.concatenate([wq, wk, wv], axis=0)               # [768, D]
        wc_t = np.ascontiguousarray(wc.T.astype(np.float16))    # [D, 768]
        ow_c = np.ascontiguousarray(
            out_w[:, c * HPC * DH:(c + 1) * HPC * DH].T.astype(np.float16)
        )  # [512, D]
        in_maps.append(
            {
                "hidden_t": hid_t,
                "wqkv_t": wc_t,
                "outw_t": ow_c,
                "cos_t": cos_t16,
                "sin_t": sin_t,
                "rot_t": rot_t,
                "trimask": tri,
                "identity": idn,
            }
        )
    return in_maps


_PROGRAM_CACHE = {}


def _get_program(B, S, D, causal):
    key = (B, S, D, causal)
    if key not in _PROGRAM_CACHE:
        _PROGRAM_CACHE[key] = build_program(B, S, D, causal=causal)
    return _PROGRAM_CACHE[key]


def _detect_causal(attention_mask, B, S):
    causal = np.triu(
        np.full((S, S), np.finfo(np.float32).min, dtype=np.float32), 1
    )
    am = np.asarray(attention_mask)
    if am.shape == (B, 1, S, S):
        if np.array_equal(am, np.broadcast_to(causal[None, None], (B, 1, S, S))):
            return True
        if not am.any():
            return False
    raise ValueError(
        "kernel only supports the causal mask from setup_inputs() or an "
        "all-zero mask"
    )


def kernel(hidden_states, position_ids, attention_mask, Wqkv_w, out_w):
    hidden_states = np.asarray(hidden_states)
    position_ids = np.asarray(position_ids)
    Wqkv_w = np.asarray(Wqkv_w)
    out_w = np.asarray(out_w)

    B, S, D = hidden_states.shape
    causal = _detect_causal(attention_mask, B, S)
    nc = _get_program(B, S, D, causal)
    in_maps = make_host_inputs(
        hidden_states, position_ids, Wqkv_w, out_w, B, S, D
    )
    res = run_bass_kernel_spmd(nc, in_maps, list(range(N_CORES)))
    out = res.results[0]["out_partial"].astype(np.float32)
    for c in range(1, N_CORES):
        out += res.results[c]["out_partial"].astype(np.float32)
    # out is [MT, OH2, 128, OW2] tiled; reassemble to [B, S, D]
    mt, oh2, _, ow2 = out.shape
    out = out.transpose(0, 2, 1, 3).reshape(B, S, D)
    return out.astype(np.float32)
